# revision 38
# baseline (speedup 1.0000x reference)
"""Trainium2 Bass kernel for nn_MLPFusionLoRA (MoE-routed fused MLP + LoRA).

Sharding: B (batch-per-modality) axis across the 8 NeuronCores — core b gets
sample b of all 4 modalities (the masked routing combine mixes modalities at
fixed b, so each core is self-contained; weights replicated).

Per-core math (feature-major layout, tokens on the matmul free dim):
  x1_i  = fc1_w @ x_i^T                                  [3072, T]
  t_e   = a1_w[e] @ x_e^T  (rank 16, zero-padded to 32)  [128, T]
  w[i,t,e] = E_ie * mask[e]*mask[i] / sum_e (mask_e+1e-6) E_ie
  x1_i += B1^T.T @ (t * wexp_i)     <- routing combine folded into one matmul
  h_i   = gelu(x1_i + fc1_b)
  y_i   = fc2_w @ h_i + B2^T.T @ (u * wexp_i) + fc2_b,  u_e = a2_w[e] @ h_e

All heavy matmuls bf16 (fp32 PSUM accumulation). The 32-wide LoRA
projections (t1, u) run in a single col-tiled PE window per tile with the
4 accumulation chains interleaved across distinct 32-col tile positions
(disjoint psum partitions) so the HW executes them ~4x concurrent. The
tiny routing matmuls (denominator, reciprocal broadcast, rank-row
broadcast) are bf16 and serve modality PAIRS via 2T-wide tiles.
"""

from contextlib import ExitStack

import numpy as np
import ml_dtypes

import concourse.bacc as bacc
import concourse.mybir as mybir
import concourse.tile as tile
from concourse import bass_utils
from concourse.bass import ds, ts

F32 = mybir.dt.float32
F32R = mybir.dt.float32r
BF16 = mybir.dt.bfloat16
NPBF = ml_dtypes.bfloat16

M, B, NT, C, H = 4, 8, 1024, 768, 3072
CK, HK = C // 128, H // 128  # 6, 24
T = 256                      # token tile
NTT = NT // T                # 4
AF = mybir.ActivationFunctionType
ALU = mybir.AluOpType

_CACHE = {}


def _build_program(nt=NT, gelu=AF.Gelu):
    ntt = nt // T
    nc = bacc.Bacc("TRN2", target_bir_lowering=False, debug=False)

    dp = lambda name, shape, dt: nc.dram_tensor(name, shape, dt, kind="ExternalInput").ap()
    xt = dp("xt", [ntt, 128, M * CK * T], BF16)    # xt[tt,p,(m c t)] = x[m,b,tt*T+t,128c+p]; one contiguous DMA per tile
    w1 = dp("w1", [HK, 128, CK * 128], BF16)       # j-major: w1[j,p,128c+o] = fc1_w[128j+o,128c+p]
    w2 = dp("w2", [HK, 128, C], BF16)              # w2[k,p,c] = fc2_w[c,128k+p]
    a1 = dp("a1", [128, CK * 128], BF16)           # pre-transposed: [p, c*128 + (32e: 4 gate cols, +4 ranks)]
    a2 = dp("a2", [128, HK * 128], BF16)           # pre-transposed: [p, k*128 + 32e+4+r]
    b1 = dp("b1", [128, H], BF16)                  # b1[32e+r,h] = b1_w[e,h,r]
    b2 = dp("b2", [128, C], BF16)                  # b2[32e+r,c] = b2_w[e,c,r]
    gb = dp("gb", [4, 4], F32)                     # gb[e,i] = gate_b[i,e] (Exp bias)
    f1b = dp("f1b", [128, HK], F32)                # fc1_b[128j+p] at [p,j]
    f2b = dp("f2b", [128, CK], F32)                # fc2_b[128j+p] at [p,j]
    mv1 = dp("mv1", [4, 1], BF16)                   # mask[e,b] + 1e-6 (denominator weights)
    selm = dp("selm", [4, 128], BF16)               # [e, 32e'+4+r] = mask[e'] * (e==e'), i-independent
    msc = dp("msc", [4, M], F32)                    # msc[:, i] = mask[i] (per-partition scalar)
    on4 = dp("on4", [1, 4], BF16)                   # ones
    yt = nc.dram_tensor("yt", [ntt, M, CK, 128, T], BF16, kind="ExternalOutput").ap()

    with tile.TileContext(nc) as tc, ExitStack() as ctx:
        wp = ctx.enter_context(tc.tile_pool(name="wts", bufs=1))
        xp = ctx.enter_context(tc.tile_pool(name="xin", bufs=2))
        hp = ctx.enter_context(tc.tile_pool(name="hts", bufs=4))
        sp = ctx.enter_context(tc.tile_pool(name="smal", bufs=2))
        wx = ctx.enter_context(tc.tile_pool(name="wexp", bufs=4))
        syp = ctx.enter_context(tc.tile_pool(name="yout", bufs=2))
        ssp = ctx.enter_context(tc.tile_pool(name="sS", bufs=4))
        pmm = ctx.enter_context(tc.tile_pool(name="pmm", bufs=4, space="PSUM"))
        ptu = ctx.enter_context(tc.tile_pool(name="ptu", bufs=2, space="PSUM"))
        prt = ctx.enter_context(tc.tile_pool(name="prt", bufs=2, space="PSUM"))

        # ---- resident weights ----
        w1s = wp.tile([128, CK * H], BF16)
        w2s = wp.tile([128, HK * C], BF16)
        a1s = wp.tile([128, CK * 128], BF16)
        a2s = wp.tile([128, HK * 128], BF16)
        b1s = wp.tile([128, H], BF16)
        b2s = wp.tile([128, C], BF16)
        gbs = wp.tile([4, 4], F32)
        nc.sync.dma_start(gbs[:], gb[:])
        f1bs = wp.tile([128, HK], F32)
        f2bs = wp.tile([128, CK], F32)
        mv1s = wp.tile([4, 1], BF16)
        nc.sync.dma_start(mv1s[:], mv1[:])
        selms = wp.tile([4, 128], BF16)
        nc.sync.dma_start(selms[:], selm[:])
        mscs = wp.tile([4, M], F32)
        nc.sync.dma_start(mscs[:], msc[:])
        ones4 = wp.tile([1, 4], BF16)
        nc.sync.dma_start(ones4[:], on4[:])

        # ---------- software-pipelined tile loop ----------
        # stage helpers keep per-tile state in dicts; tile tt+1's routing
        # (PE-light, chain-latency-heavy) is emitted inside tile tt's fc2
        # stream so the PE never waits on the ACT/DVE routing chains.

        def load_x(tt):
            xs = xp.tile([128, M * CK * T], BF16, tag="xs", name=f"xs_{tt}")
            if tt == 0:
                nc.sync.dma_start(a1s[:], a1[:])  # contiguous, tiny: unblocks t1
            # contiguous layout, split into 8 transfers for DMA-queue
            # parallelism; each modality's first c-chunks (q even) land
            # first so the c-outer t1 window unblocks earliest
            xq = M * CK * T // 8
            for q in (0, 2, 4, 6, 1, 3, 5, 7):
                nc.sync.dma_start(xs[:, q * xq:(q + 1) * xq], xt[tt, :, q * xq:(q + 1) * xq])
            if tt == 0:
                # fc1-stage weights right behind tile-0 x (j-major so the
                # first fc1 chains start after small transfers), fc2 after
                for j in range(6):
                    nc.sync.dma_start(w1s[:, j * CK * 128:(j + 1) * CK * 128], w1[j])
                nc.sync.dma_start(b1s[:], b1[:])
                nc.sync.dma_start(f1bs[:], f1b[:])
                for j in range(6, HK):
                    nc.sync.dma_start(w1s[:, j * CK * 128:(j + 1) * CK * 128], w1[j])
                for k in range(HK):
                    nc.sync.dma_start(w2s[:, k * C:(k + 1) * C], w2[k])
                nc.sync.dma_start(a2s[:], a2[:])
                nc.sync.dma_start(b2s[:], b2[:])
                nc.sync.dma_start(f2bs[:], f2b[:])
            st = {"tt": tt, "xs": xs,
                  "xv": lambda m, c, _x=xs: _x[:, (m * CK + c) * T:(m * CK + c + 1) * T]}
            return st

        def emit_lg_t1(st):
            # gate logits ride inside the a1 matmul: block e of a1 carries
            # gate_w[e] in cols 32e+0..3 and a1_w[e] ranks in cols 32e+4..19,
            # so t1 psum rows 32i..32i+3 are modality-i gate logits.
            # c-outer/e-inner: the 4 expert chains sit on distinct 32-col
            # PE tile positions (disjoint psum partitions), so the HW runs
            # them concurrently -> ~4x faster than e-outer serial chains.
            tt, xv = st["tt"], st["xv"]
            t1 = ptu.tile([128, T], F32, tag="tu", name=f"t1_{tt}")
            for c in range(CK):
                for e in range(M):
                    nc.tensor.matmul(t1[32 * e:32 * e + 32, :],
                                     a1s[:, c * 128 + 32 * e: c * 128 + 32 * e + 32],
                                     xv(e, c), start=(c == 0), stop=(c == CK - 1),
                                     tile_position=(0, 32 * e))
            st["t1"] = t1

        def emit_chains_a(st):
            # Exp on ACT. mv1 carries (mask + 1e-6) so one [4->1] matmul
            # IS the full denominator: den_i = sum_e (mask_e+1e-6) E_ie ==
            # masked-sum + 1e-6*full-sum (exact algebra of the reference's
            # renormalization). Modalities are processed in pairs (i, i+1)
            # with 2T-wide tiles so each small matmul serves both.
            tt = st["tt"]
            t1s = sp.tile([128, T], F32, tag="t1s", name=f"t1s_{tt}")
            nc.vector.tensor_copy(t1s[:], st["t1"][:])
            st["t1s"] = t1s
            Es, rv32s = [], []
            for p in range(2):
                Ecat = sp.tile([4, 2 * T], BF16, tag="Ei", bufs=2, name=f"E_{tt}_{p}")
                for h in range(2):
                    nc.scalar.activation(Ecat[:, h * T:(h + 1) * T],
                                         st["t1"][32 * (2 * p + h):32 * (2 * p + h) + 4, :],
                                         AF.Exp, bias=gbs[:, 2 * p + h:2 * p + h + 1])
                Es.append(Ecat)
            for p in range(2):
                dz = prt.tile([1, 2 * T], F32, tag="rt", name=f"dz_{tt}_{p}")
                nc.tensor.matmul(dz[:], mv1s[:, 0:1], Es[p][:], start=True, stop=True)
                rv32 = sp.tile([1, 2 * T], F32, tag="rv32", bufs=2, name=f"rv32_{tt}_{p}")
                nc.vector.reciprocal_approx_fast(rv32[:], dz[:])
                rv32s.append(rv32)
            st["Es"], st["rv32s"] = Es, rv32s

        def emit_chains_b(st):
            tt = st["tt"]
            rvs = []
            for p in range(2):
                rv = sp.tile([1, 2 * T], BF16, tag="rv", bufs=2, name=f"rv_{tt}_{p}")
                nc.vector.tensor_copy(rv[:], st["rv32s"][p][:])
                rvs.append(rv)
            wfs = []
            for p in range(2):
                rb = prt.tile([4, 2 * T], F32, tag="rt", name=f"rb_{tt}_{p}")
                nc.tensor.matmul(rb[:], ones4[:], rvs[p][:], start=True, stop=True)
                # wf_i = (E_i * mask_i) * (1/den_i); mask_i enters here so
                # the wexp broadcast weights are modality-independent
                wf = sp.tile([4, 2 * T], BF16, tag="wf", bufs=2, name=f"wf_{tt}_{p}")
                for h in range(2):
                    i = 2 * p + h
                    nc.vector.scalar_tensor_tensor(
                        wf[:, h * T:(h + 1) * T], st["Es"][p][:, h * T:(h + 1) * T],
                        mscs[:, i:i + 1], rb[:, h * T:(h + 1) * T],
                        ALU.mult, ALU.mult)
                wfs.append(wf)
            wexps = []
            for p in range(2):
                wexp_ps = prt.tile([128, 2 * T], F32, tag="rt", name=f"wexp_ps_{tt}_{p}")
                nc.tensor.matmul(wexp_ps[:], selms[:], wfs[p][:], start=True, stop=True)
                wexp = wx.tile([128, 2 * T], F32, tag="wexp", bufs=2, name=f"wexp_{tt}_{p}")
                nc.vector.tensor_copy(wexp[:], wexp_ps[:])
                wexps.append(wexp)
            st["wexps"] = wexps

        def emit_S(st):
            tt = st["tt"]
            Ss = []
            for i in range(M):
                S = ssp.tile([128, T], BF16, tag="S1", name=f"S_{tt}_{i}")
                wx_p = st["wexps"][i // 2]
                nc.vector.tensor_tensor(S[:], st["t1s"][:],
                                        wx_p[:, (i % 2) * T:(i % 2 + 1) * T], ALU.mult)
                Ss.append(S)
            st["Ss"] = Ss

        def emit_fc1(st):
            tt, xv = st["tt"], st["xv"]
            hs = []
            for i in range(M):
                hsi = hp.tile([128, HK * T], BF16, tag="hs", name=f"hs_{tt}_{i}")
                hs.append(hsi)
                for j in range(HK):
                    x1 = pmm.tile([128, T], F32, tag="mm", name=f"x1_{tt}_{i}_{j}")
                    for c in range(CK):
                        nc.tensor.matmul(x1[:], w1s[:, (j * CK + c) * 128:(j * CK + c + 1) * 128],
                                         xv(i, c), start=(c == 0), stop=False)
                    nc.tensor.matmul(x1[:], b1s[:, 128 * j:128 * (j + 1)], st["Ss"][i][:],
                                     start=False, stop=True)
                    nc.scalar.activation(hsi[:, j * T:(j + 1) * T], x1[:], gelu,
                                         bias=f1bs[:, j:j + 1])
            st["hs"] = hs

        def emit_u(st):
            # a2 projections for all 4 modalities, k-outer/i-inner: the 4
            # accumulation chains occupy distinct 32-col PE tile positions
            # and disjoint psum partitions, so they run ~4x concurrent.
            tt = st["tt"]
            u = ptu.tile([128, T], F32, tag="tu", name=f"u_{tt}")
            hs = st["hs"]
            for j in range(HK):
                for i in range(M):
                    nc.tensor.matmul(u[32 * i:32 * i + 32, :],
                                     a2s[:, j * 128 + 32 * i: j * 128 + 32 * i + 32],
                                     hs[i][:, j * T:(j + 1) * T],
                                     start=(j == 0), stop=(j == HK - 1),
                                     tile_position=(0, 32 * i))
            st["u"] = u

        def emit_us_S2(st):
            tt = st["tt"]
            us = sp.tile([128, T], F32, tag="us", name=f"us_{tt}")
            nc.vector.tensor_copy(us[:], st["u"][:])
            S2s = []
            for i in range(M):
                S2 = ssp.tile([128, T], BF16, tag="S2", name=f"S2_{tt}_{i}")
                wx_p = st["wexps"][i // 2]
                nc.vector.tensor_tensor(S2[:], us[:],
                                        wx_p[:, (i % 2) * T:(i % 2 + 1) * T], ALU.mult)
                S2s.append(S2)
            st["S2s"] = S2s

        def emit_fc2(st, i_list):
            tt = st["tt"]
            for i in i_list:
                for j in range(CK):
                    y = pmm.tile([128, T], F32, tag="mm", name=f"y_{tt}_{i}_{j}")
                    for k in range(HK):
                        nc.tensor.matmul(y[:], w2s[:, k * C + 128 * j: k * C + 128 * (j + 1)],
                                         st["hs"][i][:, k * T:(k + 1) * T],
                                         start=(k == 0), stop=False)
                    nc.tensor.matmul(y[:], b2s[:, 128 * j:128 * (j + 1)], st["S2s"][i][:],
                                     start=False, stop=True)
                    ysb = syp.tile([128, T], BF16, tag="y", name=f"ysb_{tt}_{i}_{j}")
                    nc.vector.tensor_scalar_add(ysb[:], y[:], f2bs[:, j:j + 1])
                    nc.sync.dma_start(yt[tt, i, j], ysb[:])

        st = load_x(0)
        emit_lg_t1(st)
        emit_chains_a(st)
        emit_chains_b(st)
        emit_S(st)
        for tt in range(ntt):
            emit_fc1(st)
            # one contiguous (128,32)-tiled PE window: u(tt) then t1(tt+1)
            emit_u(st)
            nxt = None
            if tt + 1 < ntt:
                nxt = load_x(tt + 1)
                emit_lg_t1(nxt)
            emit_us_S2(st)
            emit_fc2(st, [0])
            if nxt is not None:
                emit_chains_a(nxt)
            emit_fc2(st, [1])
            if nxt is not None:
                emit_chains_b(nxt)
            emit_fc2(st, [2, 3])
            if nxt is not None:
                emit_S(nxt)
                st = nxt

    nc.compile()
    return nc


def _prep_inputs(x, modality_mask, fc1_w, fc1_b, fc2_w, fc2_b, gate_w, gate_b,
                 a1_w, b1_w, a2_w, b2_w):
    """Build the 8 per-core input maps (numpy, host-side layout prep)."""
    bf = lambda a: np.ascontiguousarray(a).astype(NPBF)
    f32 = lambda a: np.ascontiguousarray(a, dtype=np.float32)

    xm = np.asarray(x, np.float32).reshape(M, B, NTT, T, CK, 128)
    # xt[b][tt, p, (m c t)] = x[m,b,tt*T+t,128c+p]: SBUF-exact layout so each
    # tile's x arrives as one fully-contiguous DMA (12KB lines)
    xt_all = bf(xm.transpose(1, 2, 5, 0, 4, 3).reshape(B, NTT, 128, M * CK * T))

    # j-major: w1h[j, p, 128c+o] = fc1_w[128j+o, 128c+p]
    w1h = bf(np.asarray(fc1_w, np.float32).reshape(HK, 128, CK, 128)
             .transpose(0, 3, 2, 1).reshape(HK, 128, CK * 128))
    w2h = bf(np.asarray(fc2_w, np.float32).T.reshape(HK, 128, C))
    # a1[c,p,32e+r] = a1_w[e,r,128c+p]
    a1p = np.zeros((CK, 128, 128), np.float32)
    a1t = np.asarray(a1_w, np.float32).transpose(2, 0, 1).reshape(CK, 128, M, 16)
    gwt = np.asarray(gate_w, np.float32).transpose(2, 0, 1).reshape(CK, 128, M, M)
    for e in range(M):
        # cols 32e+0..3: gate_w[e] (modality-e logits); cols 32e+4..19: ranks
        a1p[:, :, 32 * e:32 * e + 4] = gwt[:, :, e, :]
        a1p[:, :, 32 * e + 4:32 * e + 20] = a1t[:, :, e, :]
    a2p = np.zeros((HK, 128, 128), np.float32)
    a2t = np.asarray(a2_w, np.float32).transpose(2, 0, 1).reshape(HK, 128, M, 16)
    for e in range(M):
        a2p[:, :, 32 * e + 4:32 * e + 20] = a2t[:, :, e, :]
    b1p = np.zeros((128, H), np.float32)
    b2p = np.zeros((128, C), np.float32)
    b1t = np.asarray(b1_w, np.float32).transpose(0, 2, 1)  # [e, r, h]
    b2t = np.asarray(b2_w, np.float32).transpose(0, 2, 1)  # [e, r, c]
    for e in range(M):
        b1p[32 * e + 4:32 * e + 20, :] = b1t[e]
        b2p[32 * e + 4:32 * e + 20, :] = b2t[e]
    gbh = f32(np.asarray(gate_b, np.float32).T)  # [e, i]
    f1bh = f32(np.asarray(fc1_b, np.float32).reshape(HK, 128).T)
    f2bh = f32(np.asarray(fc2_b, np.float32).reshape(CK, 128).T)

    maskf = np.asarray(modality_mask, np.float32)  # [M(e), B]
    a1h = a1p.transpose(1, 0, 2).reshape(128, CK * 128)
    a2h = a2p.transpose(1, 0, 2).reshape(128, HK * 128)
    shared = dict(w1=w1h, w2=w2h, a1=bf(a1h), a2=bf(a2h), b1=bf(b1p), b2=bf(b2p),
                  gb=gbh, f1b=f1bh, f2b=f2bh)

    in_maps = []
    for b in range(B):
        mb = maskf[:, b]  # mask[e]
        mv1 = (mb + 1e-6).astype(np.float32).reshape(4, 1)
        selmb = np.zeros((4, 128), np.float32)
        for e in range(M):
            selmb[e, 32 * e + 4:32 * e + 20] = mb[e]
        mscb = np.tile(mb.reshape(1, 4), (4, 1))  # msc[:, i] = mask[i]
        in_maps.append(dict(shared, xt=xt_all[b], mv1=bf(mv1), selm=bf(selmb),
                            msc=f32(mscb), on4=bf(np.ones((1, 4), np.float32))))
    return in_maps


def _run(inputs, trace=False, trace_kwargs=None):
    if "nc" not in _CACHE:
        _CACHE["nc"] = _build_program()
    nc = _CACHE["nc"]
    in_maps = _prep_inputs(**inputs)
    kw = {}
    if trace:
        kw = dict(trace=True, trace_kwargs=trace_kwargs or {})
    res = bass_utils.run_bass_kernel_spmd(nc, in_maps, list(range(B)), **kw)
    # yt[m,c,p,t] -> y[m*B+b, t, 128c+p]
    y = np.empty((M * B, NT, C), np.float32)
    for b in range(B):
        ytb = np.asarray(res.results[b]["yt"], dtype=np.float32)  # [NTT, M, CK, 128, T]
        # y[m*B+b, tt*T+t, 128c+p] = ytb[tt, m, c, p, t]
        y[b::B] = ytb.transpose(1, 0, 4, 2, 3).reshape(M, NT, C)
    return y, res


def kernel(**inputs):
    y, _ = _run(inputs)
    return y

